# revision 1
# baseline (speedup 1.0000x reference)
"""GQA forward kernel for Trainium2, 8-core tensor-parallel (group-aligned).

Problem: B=2, T=2048, D=2048, 32 Q heads / 8 KV heads, head_dim 64, causal.

Sharding: core c owns KV head c and its 4 Q heads (whole GQA group), both
batches.  Output projection is row-parallel Megatron style: each core
contracts its 256 attention-output channels against its slice of Wo and the
host sums the 8 partial outputs (+ bo).

Device-side dataflow per core (matmuls in float32r unless noted, fp32 accum):
  x^T [C, T] (host-transposed)
    -> QKK proj:  lhsT = [Wq_c | Wk_c | Wk_c]  -> Q^T [256, T], K^T dup [128, T]
    -> V proj (fp16): lhsT = x^T fp16, rhs = Wv_c fp16 -> V [T, 64] natural
  attention per (batch, head-pair, q-chunk of 512), scores TRANSPOSED:
    S^T[kv, q] = matmul(lhsT=K^T tile [64,128], rhs=Q^T [64, 512])
      head pairs run on disjoint PE row groups (base partitions 0 / 64)
    expS = ACT Exp(S^T / 8)  (no max-subtraction: |scores| <= ~6)
    causal: column-sliced matmuls + one triangle mask on diagonal tiles
    AV: matmul(lhsT=V2 [kv,65] (V plus ones col), rhs=expS) accumulated over
        kv tiles -> [attn^T; den] in PSUM
    normalize: den replicated to 64 partitions via K=1 ones-matmul,
        reciprocal + multiply on DVE
  out-proj: y[t, e] = matmul(lhsT=attn^T [256, t], rhs=Wo_c [256, e])
"""

import os

import numpy as np

import concourse.mybir as mybir
import concourse.tile as tile
from concourse import bacc
from concourse import bass_utils

P = 128
B = 2
T = 2048
C = 2048
HD = 64
QH = 32
KVH = 8
G = QH // KVH  # 4
NCORES = 8
QH_LOC = QH // NCORES  # 4 q heads per core
TCH = 256  # token chunk for projection phase
QCH = 512  # q chunk for attention phase
KT = C // P  # 16 contraction tiles
f32 = mybir.dt.float32
f32r = mybir.dt.float32r
bf16 = mybir.dt.bfloat16
fp16 = mybir.dt.float16

_CACHE = {}


def _build():
    nc = bacc.Bacc("TRN2", target_bir_lowering=False, debug=False, num_devices=NCORES)

    xt = nc.dram_tensor("xt", [B, C, T], f32, kind="ExternalInput")
    xtb = nc.dram_tensor("xtb", [B, C, T], fp16, kind="ExternalInput")
    wqk = nc.dram_tensor("wqk", [C, 384], f32, kind="ExternalInput")
    wv = nc.dram_tensor("wv", [C, HD], fp16, kind="ExternalInput")
    wo = nc.dram_tensor("wo", [G * HD, C], f32, kind="ExternalInput")
    bqk = nc.dram_tensor("bqk", [P, 3], f32, kind="ExternalInput")
    bv = nc.dram_tensor("bv", [1, HD], f32, kind="ExternalInput")
    maskd = nc.dram_tensor("mask", [P, P], f32, kind="ExternalInput")
    y = nc.dram_tensor("y", [B, T, C], f32, kind="ExternalOutput")

    wqk3 = wqk.ap().rearrange("(ko p) m -> p ko m", p=P).bitcast(f32r)
    wv3 = wv.ap().rearrange("(ko p) m -> p ko m", p=P)
    wo3 = wo.ap().rearrange("(ko p) m -> p ko m", p=P).bitcast(f32r)

    with tile.TileContext(nc) as tc:
        with (
            tc.tile_pool(name="const", bufs=1) as cpool,
            tc.tile_pool(name="x", bufs=2) as xpool,
            tc.tile_pool(name="proj", bufs=1) as projpool,
            tc.tile_pool(name="attn", bufs=1) as apool,
            tc.tile_pool(name="work", bufs=5) as wpool,
            tc.tile_pool(name="work2", bufs=6) as wpool2,
            tc.tile_pool(name="psA", bufs=2, space="PSUM") as psumA,
            tc.tile_pool(name="psB", bufs=2, space="PSUM") as psumB,
            tc.tile_pool(name="psC", bufs=2, space="PSUM") as psumC,
        ):
            # ---- constants / weights (resident) ----
            # startup-critical DMA order: wqk sub0, then x chunk 0 (the first
            # 16 QKK matmuls need only these), then the rest
            wqk_sb = cpool.tile([P, KT, 384], f32r)
            nc.sync.dma_start(wqk_sb[:, :, 0:P], wqk3[:, :, 0:P])
            xb0 = xt.ap()[0].rearrange("(ko p) t -> p ko t", p=P).bitcast(f32r)
            xbb0 = xtb.ap()[0].rearrange("(ko p) t -> p ko t", p=P)
            xch0 = xpool.tile([P, KT, TCH], f32r, tag="xch", name="xch")
            nc.sync.dma_start(xch0[:, 0 : KT // 2, :], xb0[:, 0 : KT // 2, 0:TCH])
            nc.sync.dma_start(xch0[:, KT // 2 :, :], xb0[:, KT // 2 :, 0:TCH])
            for _s in range(1, 3):
                nc.sync.dma_start(
                    wqk_sb[:, :, _s * P : (_s + 1) * P], wqk3[:, :, _s * P : (_s + 1) * P]
                )
            xchb0 = xpool.tile([P, KT, TCH], fp16, tag="xchb", name="xchb")
            nc.sync.dma_start(xchb0[:, 0 : KT // 2, :], xbb0[:, 0 : KT // 2, 0:TCH])
            nc.sync.dma_start(xchb0[:, KT // 2 :, :], xbb0[:, KT // 2 :, 0:TCH])
            wv_sb = cpool.tile([P, KT, HD], fp16)
            nc.sync.dma_start(wv_sb[:], wv3)
            bqk_sb = cpool.tile([P, 3], f32)
            nc.sync.dma_start(bqk_sb[:], bqk.ap())
            bv_sb = cpool.tile([P, HD], f32)
            nc.sync.dma_start(bv_sb[:], bv.ap().to_broadcast((P, HD)))
            mask_sb = cpool.tile([P, P], f32r)
            nc.sync.dma_start(mask_sb[:], maskd.ap().bitcast(f32r))
            ones_f32 = cpool.tile([P, KT], f32)
            nc.gpsimd.memset(ones_f32[:], 1.0)
            ones_r = cpool.tile([P, HD], f32r)
            nc.vector.tensor_copy(ones_r[:], ones_f32[:, 0:1].to_broadcast((P, HD)))
            wo_sb = cpool.tile([P, 2, C], f32r)

            def emit_p3(pb, pattn, pqc):
                for ts in range(pqc * (QCH // P), (pqc + 1) * (QCH // P)):
                    for ec in range(C // QCH):
                        py = psumC.tile([P, QCH], f32, tag="pp", name="py")
                        for ks in range(2):
                            nc.tensor.matmul(
                                py[:],
                                pattn[:, ks, ts * P : (ts + 1) * P],
                                wo_sb[:, ks, ec * QCH : (ec + 1) * QCH],
                                start=(ks == 0),
                                stop=(ks == 1),
                            )
                        y_sb = wpool2.tile([P, QCH], f32, tag="ysb")
                        nc.any.tensor_copy(y_sb[:], py[:])
                        nc.sync.dma_start(
                            y.ap()[
                                pb, ts * P : (ts + 1) * P, ec * QCH : (ec + 1) * QCH
                            ],
                            y_sb[:],
                        )

            deferred_p3 = None
            for b in range(B):
                xb = xt.ap()[b].rearrange("(ko p) t -> p ko t", p=P).bitcast(f32r)
                xbb = xtb.ap()[b].rearrange("(ko p) t -> p ko t", p=P)

                # ---- P1: projections ----
                qkk_sb = projpool.tile([P, 3, T], f32r, tag="qkk")
                v2_sb = projpool.tile([P, KT, 130], f32r, tag="v2")
                nc.vector.tensor_copy(v2_sb[:, :, 64:65], ones_f32[:, :, None])
                for tch in range(T // TCH):
                    tsl = slice(tch * TCH, (tch + 1) * TCH)
                    if b == 0 and tch == 0:
                        xch, xchb = xch0, xchb0
                    else:
                        xch = xpool.tile([P, KT, TCH], f32r, tag="xch", name="xch")
                        nc.sync.dma_start(xch[:, 0 : KT // 2, :], xb[:, 0 : KT // 2, tsl])
                        nc.sync.dma_start(xch[:, KT // 2 :, :], xb[:, KT // 2 :, tsl])
                        xchb = xpool.tile([P, KT, TCH], fp16, tag="xchb", name="xchb")
                        nc.sync.dma_start(xchb[:, 0 : KT // 2, :], xbb[:, 0 : KT // 2, tsl])
                        nc.sync.dma_start(xchb[:, KT // 2 :, :], xbb[:, KT // 2 :, tsl])
                    if tch == 4 and b == 0:
                        nc.sync.dma_start(wo_sb[:], wo3)
                    if tch == 3 and deferred_p3 is not None:
                        emit_p3(*deferred_p3)
                        deferred_p3 = None
                    for sub in range(3):
                        pp_full = psumC.tile([P, QCH], f32, tag="pp", name="pp")
                        pp = pp_full[:, :TCH]
                        for k in range(KT):
                            nc.tensor.matmul(
                                pp[:],
                                wqk_sb[:, k, sub * P : (sub + 1) * P],
                                xch[:, k, :],
                                start=(k == 0),
                                stop=(k == KT - 1),
                            )
                        nc.any.tensor_tensor(
                            qkk_sb[:, sub, tsl],
                            pp[:],
                            bqk_sb[:, sub : sub + 1].to_broadcast((P, TCH)),
                            mybir.AluOpType.add,
                        )
                    for ts in range(TCH // P):
                        tidx = tch * (TCH // P) + ts
                        pv = psumC.tile([P, HD], f32, tag="pp", name="pv")
                        for k in range(KT):
                            nc.tensor.matmul(
                                pv[:],
                                xchb[:, k, ts * P : (ts + 1) * P],
                                wv_sb[:, k, :],
                                start=(k == 0),
                                stop=(k == KT - 1),
                            )
                        nc.any.tensor_tensor(
                            v2_sb[:, tidx, 0:64], pv[:], bv_sb[:], mybir.AluOpType.add
                        )
                        nc.any.tensor_tensor(
                            v2_sb[:, tidx, 65:129], pv[:], bv_sb[:], mybir.AluOpType.add
                        )

                # ---- P2 + P3 interleaved: attention then out-proj per q-chunk ----
                # Head pairs (2*sub, 2*sub+1) run QK^T on disjoint PE row
                # groups (base partitions 0 / 64); their score tiles share one
                # 2-bank PSUM tile so exp is a single wide ACT op.
                attn_sb = apool.tile([P, 2, T], f32r, tag="attn")
                for qc in range(T // QCH):
                    q0 = qc * QCH
                    nfull = q0 // P
                    ntiles = nfull + QCH // P
                    for sub in range(2):
                        qT0 = qkk_sb[0:64, sub, q0 : q0 + QCH]
                        qT1 = qkk_sb[64:128, sub, q0 : q0 + QCH]
                        pav0 = psumB.tile([P, QCH], f32, tag="pav", name="pav0")
                        pav1 = psumB.tile([P, QCH], f32, tag="pav", name="pav1")
                        for i in range(ntiles):
                            if i < nfull:
                                nsl = slice(0, QCH)
                            else:
                                nsl = slice((i - nfull) * P, QCH)
                            ksl = slice(i * P, (i + 1) * P)
                            ps_s = psumA.tile([P, 2, QCH], f32, tag="ps", name="ps_s")
                            # concurrent pair: disjoint PE row groups 0-63 / 64-127
                            nc.tensor.matmul(
                                ps_s[:, 0, nsl],
                                qkk_sb[0:64, 2, ksl],
                                qT0[:, nsl],
                                start=True,
                                stop=True,
                            )
                            nc.tensor.matmul(
                                ps_s[:, 1, nsl],
                                qkk_sb[64:128, 2, ksl],
                                qT1[:, nsl],
                                start=True,
                                stop=True,
                            )
                            expS = wpool.tile([P, 2, QCH], f32r, tag="expS")
                            nc.scalar.activation(
                                expS[:, :, nsl],
                                ps_s[:, :, nsl],
                                mybir.ActivationFunctionType.Exp,
                                scale=0.125,
                            )
                            if i >= nfull:
                                j = i - nfull
                                nc.any.tensor_tensor(
                                    expS[:, :, j * P : (j + 1) * P],
                                    expS[:, :, j * P : (j + 1) * P],
                                    mask_sb[:, None, :].to_broadcast((P, 2, P)),
                                    mybir.AluOpType.mult,
                                )
                            for half, pav in ((0, pav0), (1, pav1)):
                                nc.tensor.matmul(
                                    pav[0:65, nsl],
                                    v2_sb[:, i, 0:65],
                                    expS[:, half, nsl],
                                    start=(i == 0),
                                    stop=(i == ntiles - 1),
                                    skip_group_check=True,
                                )
                        for half, pav in ((0, pav0), (1, pav1)):
                            den_sb = wpool2.tile([P, QCH], f32r, tag="den")
                            nc.any.tensor_copy(den_sb[64:65, :], pav[64:65, :])
                            ps_den = psumA.tile([64, QCH], f32, tag="ps", name="psd")
                            nc.tensor.matmul(
                                ps_den[:],
                                ones_r[64:65, 0:64],
                                den_sb[64:65, :],
                                start=True,
                                stop=True,
                            )
                            rec = wpool2.tile([64, QCH], f32, tag="rec")
                            nc.vector.reciprocal(rec[:], ps_den[:])
                            if half == 0:
                                nc.any.tensor_tensor(
                                    attn_sb[0:64, sub, q0 : q0 + QCH],
                                    pav[0:64, :],
                                    rec[:],
                                    mybir.AluOpType.mult,
                                )
                            else:
                                alo = wpool2.tile([64, QCH], f32r, tag="alo")
                                nc.any.tensor_tensor(
                                    alo[:], pav[0:64, :], rec[:], mybir.AluOpType.mult
                                )
                                nc.sync.dma_start(
                                    attn_sb[64:128, sub, q0 : q0 + QCH], alo[:]
                                )

                    # out-proj for the finished token range; the last q-chunk is
                    # deferred into the next batch's P1 (fills PE during DMA waits)
                    if qc < T // QCH - 1 or b == B - 1:
                        emit_p3(b, attn_sb, qc)
                    else:
                        deferred_p3 = (b, attn_sb, qc)

            if deferred_p3 is not None:
                emit_p3(*deferred_p3)

    nc.compile()
    return nc


def _prep_inputs(x, Wq, bq, Wk, bk, Wv, bv, Wo, bo):
    x = np.ascontiguousarray(np.asarray(x, dtype=np.float32))
    xt = np.ascontiguousarray(x.transpose(0, 2, 1))
    xtb = xt.astype(np.float16)
    Wq = np.asarray(Wq, dtype=np.float32)
    Wk = np.asarray(Wk, dtype=np.float32)
    Wv = np.asarray(Wv, dtype=np.float32)
    Wo = np.asarray(Wo, dtype=np.float32)
    bq = np.asarray(bq, dtype=np.float32)
    bk = np.asarray(bk, dtype=np.float32)
    bv = np.asarray(bv, dtype=np.float32)

    # mask[kj, qi] = 1 iff kj <= qi  (upper triangular incl. diag)
    mask = np.triu(np.ones((P, P), dtype=np.float32)).copy()
    in_maps = []
    for c in range(NCORES):
        qs = slice(c * G * HD, (c + 1) * G * HD)
        ks = slice(c * HD, (c + 1) * HD)
        wqk_c = np.concatenate([Wq[:, qs], Wk[:, ks], Wk[:, ks]], axis=1)
        bq_c = bq[qs]
        bqk_c = np.stack(
            [bq_c[0:128], bq_c[128:256], np.concatenate([bk[ks], bk[ks]])], axis=1
        )
        in_maps.append(
            {
                "xt": xt,
                "xtb": xtb,
                "wqk": np.ascontiguousarray(wqk_c),
                "wv": np.ascontiguousarray(Wv[:, ks]).astype(np.float16),
                "wo": np.ascontiguousarray(Wo[qs, :]),
                "bqk": np.ascontiguousarray(bqk_c),
                "bv": np.ascontiguousarray(bv[None, ks]),
                "mask": mask,
            }
        )
    return in_maps


def kernel(x, Wq, bq, Wk, bk, Wv, bv, Wo, bo, _trace=False):
    # NTFF tracing is unavailable through this axon client; make sure a
    # stray BASS_TRACE=1 in the environment cannot divert the run path.
    if not _trace:
        os.environ["BASS_NEVER_TRACE"] = "1"
    if "nc" not in _CACHE:
        _CACHE["nc"] = _build()
    nc = _CACHE["nc"]
    in_maps = _prep_inputs(x, Wq, bq, Wk, bk, Wv, bv, Wo, bo)
    res = bass_utils.run_bass_kernel_spmd(
        nc, in_maps, core_ids=list(range(NCORES)), trace=_trace
    )
    bo = np.asarray(bo, dtype=np.float32)
    y = np.zeros((B, T, C), dtype=np.float32)
    for c in range(NCORES):
        y += res.results[c]["y"]
    y += bo
    if _trace:
        return y, res
    return y



# revision 2
# speedup vs baseline: 1.0058x; 1.0058x over previous
"""GQA forward kernel for Trainium2, 8-core (batch x head-quarter) parallel.

Problem: B=2, T=2048, D=2048, 32 Q heads / 8 KV heads, head_dim 64, causal.

Sharding: core c = (batch b = c//4, group g = c%4) owns batch b and head
group g: q heads 8g..8g+7 (as 4 pairs p = heads 8g+p | 8g+4+p), kv heads
2g, 2g+1.  Each core emits a full [T, C] fp16 partial output (row-parallel
over its 512 attn channels); host sums 4 partials per batch (+ bo).

All-fp16 matmul path (fp32 PSUM accumulation):
  P1 projections -> Q^T pair-subs [128,T], K^T [128,T], V natural [T,2,64].
  P2 scores S^T[kv,q] per pair: two K=64 matmuls on partition halves into
     separate PSUM banks, one wide exp (ACT, scale 1/8, no max-subtract),
     causal via column-sliced matmuls + mask multiply on diagonal tiles.
  AV natural: [q,65] += expS.T @ [V|1] per head (denominator = 65th col),
     reciprocal + free-broadcast multiply (DVE), PE transpose (fp16
     identity) -> attn^T.
  P3 y = attn^T.T @ Wo_c (4 k-subs), fp16 y, per-512-col DMA.

Scheduling: PE is in-order, and phase A (scores+exp) is ACT-bound, so all
other PE work is emitted as units drained between score matmuls:
  hot queue  - AV/normalize/transpose/out-proj units (these recycle the
               expS/pav/psC rings; draining them promptly is required for
               correctness of ring reuse ordering - a cold-starved hot
               queue can deadlock the in-order PE stream)
  cold queue - P1 half-chain units for token chunks 2..7 (attention for
               q-chunk qc only needs projections through chunk 2qc+1,
               which gate_chunk() enforces before each A phase)
Each A step drains >=1 hot unit plus enough units to fill the ACT-PE gap.
"""

import os
from collections import deque

import numpy as np

import concourse.mybir as mybir
import concourse.tile as tile
from concourse import bacc
from concourse import bass_utils

P = 128
B = 2
T = 2048
C = 2048
HD = 64
NCORES = 8
NPAIR = 4
TCH = 256
KT = C // P            # 16 contraction tiles
QCH = 512
NQC = T // QCH         # 4 q chunks
NTT = T // P           # 16 token tiles
NCH = T // TCH         # 8 projection chunks
f32 = mybir.dt.float32
fp16 = mybir.dt.float16

_CACHE = {}


def _build():
    nc = bacc.Bacc("TRN2", target_bir_lowering=False, debug=False, num_devices=NCORES)

    xt = nc.dram_tensor("xt", [P, KT, T], fp16, kind="ExternalInput")
    wq = nc.dram_tensor("wq", [P, KT, 4 * P], fp16, kind="ExternalInput")
    wk = nc.dram_tensor("wk", [P, KT, P], fp16, kind="ExternalInput")
    wv = nc.dram_tensor("wv", [P, KT, P], fp16, kind="ExternalInput")
    wo = nc.dram_tensor("wo", [P, 4, C], fp16, kind="ExternalInput")
    bqk = nc.dram_tensor("bqk", [P, 5], f32, kind="ExternalInput")
    bv = nc.dram_tensor("bv", [1, P], f32, kind="ExternalInput")
    maskd = nc.dram_tensor("mask", [P, P], fp16, kind="ExternalInput")
    identd = nc.dram_tensor("ident", [P, P], fp16, kind="ExternalInput")
    y = nc.dram_tensor("y", [T, C], fp16, kind="ExternalOutput")

    with tile.TileContext(nc) as tc:
        with (
            tc.tile_pool(name="const", bufs=1) as cpool,
            tc.tile_pool(name="x", bufs=6) as xpool,
            tc.tile_pool(name="proj", bufs=1) as projpool,
            tc.tile_pool(name="exp", bufs=30) as epool,
            tc.tile_pool(name="work", bufs=6) as wpool,
            tc.tile_pool(name="y", bufs=2) as ypool,
            tc.tile_pool(name="psS", bufs=2, space="PSUM") as psS,
            tc.tile_pool(name="psAV", bufs=2, space="PSUM") as psAV,
            tc.tile_pool(name="psC", bufs=2, space="PSUM") as psC,
        ):
            # ---- DMA schedule: startup-critical quarter interleave ----
            wq_sb = cpool.tile([P, KT, 4 * P], fp16)
            xchs = [
                xpool.tile([P, KT, TCH], fp16, tag="xch", name=f"xch{i}")
                for i in range(NCH)
            ]
            qk_sb = projpool.tile([P, 5, T], fp16, tag="qk")
            v2_sb = projpool.tile([P, NTT, 2, 65], fp16, tag="v2")
            attnT = projpool.tile([P, NPAIR, T], fp16, tag="attnT")
            nc.gpsimd.memset(v2_sb[:, :, :, 64:65], 1.0)
            # SP queue: weights/biases (HWDGE); Pool queue: x/consts/wo (SWDGE)
            bqk_sb = cpool.tile([P, 5], f32)
            wk_sb = cpool.tile([P, KT, P], fp16)
            wv_sb = cpool.tile([P, KT, P], fp16)
            bv_sb = cpool.tile([P, P], f32)
            for q in range(4):
                kq = slice(4 * q, 4 * (q + 1))
                nc.sync.dma_start(wq_sb[:, kq, :], wq.ap()[:, kq, :])
                nc.gpsimd.dma_start(xchs[0][:, kq, :], xt.ap()[:, kq, 0:TCH])
                if q == 1:
                    nc.sync.dma_start(bqk_sb[:], bqk.ap())
                    nc.sync.dma_start(wk_sb[:], wk.ap())
                    nc.sync.dma_start(wv_sb[:], wv.ap())
            nc.sync.dma_start(bv_sb[:], bv.ap().to_broadcast((P, P)))
            mask_sb = cpool.tile([P, P], fp16)
            nc.gpsimd.dma_start(mask_sb[:], maskd.ap())
            id_sb = cpool.tile([P, P], fp16)
            nc.gpsimd.dma_start(id_sb[:], identd.ap())
            wo_sb = cpool.tile([P, 4, C], fp16)
            for i in list(range(1, 6)) + [-1, 6, 7]:
                if i == -1:
                    nc.gpsimd.dma_start(wo_sb[:], wo.ap())
                    continue
                tsl = slice(i * TCH, (i + 1) * TCH)
                nc.gpsimd.dma_start(xchs[i][:, 0 : KT // 2, :], xt.ap()[:, 0 : KT // 2, tsl])
                nc.gpsimd.dma_start(xchs[i][:, KT // 2 :, :], xt.ap()[:, KT // 2 :, tsl])

            # ---- P1 unit emitters ----
            chain_cells = {}

            def p1_sub_half(tch, sub, half):
                def run():
                    xch = xchs[tch]
                    if half == 0:
                        pp = psC.tile([P, QCH], f32, tag="c", name="pp")
                        chain_cells[(tch, sub)] = pp
                    else:
                        pp = chain_cells.pop((tch, sub))
                    for k in range(half * (KT // 2), (half + 1) * (KT // 2)):
                        lhsT = (
                            wq_sb[:, k, sub * P : (sub + 1) * P]
                            if sub < 4
                            else wk_sb[:, k, :]
                        )
                        nc.tensor.matmul(
                            pp[:, 0:TCH], lhsT, xch[:, k, :],
                            start=(k == 0), stop=(k == KT - 1),
                        )
                    if half == 1:
                        tsl = slice(tch * TCH, (tch + 1) * TCH)
                        nc.vector.tensor_tensor(
                            qk_sb[:, sub, tsl], pp[:, 0:TCH],
                            bqk_sb[:, sub : sub + 1].to_broadcast((P, TCH)),
                            mybir.AluOpType.add,
                        )
                return run

            def p1_v_half(tch, ts2, half):
                def run():
                    xch = xchs[tch]
                    tidx = tch * (TCH // P) + ts2
                    if half == 0:
                        pv = psC.tile([P, QCH], f32, tag="c", name="pv")
                        chain_cells[(tch, "v", ts2)] = pv
                    else:
                        pv = chain_cells.pop((tch, "v", ts2))
                    for k in range(half * (KT // 2), (half + 1) * (KT // 2)):
                        nc.tensor.matmul(
                            pv[:, 0:P], xch[:, k, ts2 * P : (ts2 + 1) * P], wv_sb[:, k, :],
                            start=(k == 0), stop=(k == KT - 1),
                        )
                    if half == 1:
                        nc.vector.tensor_tensor(
                            v2_sb[:, tidx, :, 0:64],
                            pv[:, 0:P].rearrange("p (h d) -> p h d", h=2),
                            bv_sb[:].rearrange("p (h d) -> p h d", h=2),
                            mybir.AluOpType.add,
                        )
                return run

            def p1_chunk_units(tch):
                units = []
                for sub in range(5):
                    units.append((853, p1_sub_half(tch, sub, 0)))
                    units.append((853, p1_sub_half(tch, sub, 1)))
                for ts2 in range(TCH // P):
                    units.append((427, p1_v_half(tch, ts2, 0)))
                    units.append((427, p1_v_half(tch, ts2, 1)))
                return units

            # chunks 0,1 inline (needed before any attention)
            for tch in range(2):
                for _, u in p1_chunk_units(tch):
                    u()

            hot = deque()
            warm = deque()
            cold = deque()
            chunk_done = 1  # highest P1 chunk fully emitted
            for tch in range(2, NCH):
                for cost, u in p1_chunk_units(tch):
                    cold.append((cost, u))

            cold_popped = [0]
            cold_total = {tch: 14 * (tch - 1) for tch in range(2, NCH)}

            def pop_cold():
                cost, u = cold.popleft()
                cold_popped[0] += 1
                u()
                return cost

            def gate_chunk(tch):
                # force-drain cold until chunk tch fully emitted
                nonlocal chunk_done
                if tch <= chunk_done:
                    return
                need = cold_total[min(tch, NCH - 1)] - cold_popped[0]
                for _ in range(max(0, need)):
                    pop_cold()
                chunk_done = max(chunk_done, tch)

            def drain(budget, cold_cap=10**9, warm_keep=0):
                while hot and budget > 0:
                    cost, u = hot.popleft()
                    u()
                    budget -= cost
                while warm and budget > 0 and len(warm) > warm_keep:
                    cost, u = warm.popleft()
                    u()
                    budget -= cost
                while cold and budget > 0 and cold_popped[0] < cold_cap:
                    budget -= pop_cold()

            # ---- attention unit emitters ----
            exp_tiles = {}
            pav_cells = {}
            anat_cells = {}
            y_cells = {}

            def b_unit(qc, pair, qs):
                def run():
                    nfull = 4 * qc
                    last = nfull + qs
                    pav0 = psAV.tile([P, QCH], f32, tag="av", name="pav0")
                    pav1 = psAV.tile([P, QCH], f32, tag="av", name="pav1")
                    qsl = slice(qs * P, (qs + 1) * P)
                    for i in range(last + 1):
                        e = exp_tiles[(qc, pair, i)]
                        nc.tensor.matmul(
                            pav0[:, 0:65], e[:, 0, qsl], v2_sb[:, i, 0, :],
                            start=(i == 0), stop=(i == last), skip_group_check=True,
                        )
                        nc.tensor.matmul(
                            pav1[:, 0:65], e[:, 1, qsl], v2_sb[:, i, 1, :],
                            start=(i == 0), stop=(i == last), skip_group_check=True,
                        )
                    pav_cells[(qc, pair, qs)] = (pav0, pav1)
                return run

            def c1_unit(qc, pair, qs):
                def run():
                    pav0, pav1 = pav_cells.pop((qc, pair, qs))
                    rec = wpool.tile([P, 2, 1], f32, tag="rec")
                    anat = wpool.tile([P, 2, 64], fp16, tag="anat")
                    for h, pav in ((0, pav0), (1, pav1)):
                        nc.vector.reciprocal(rec[:, h, :], pav[:, 64:65])
                        nc.vector.tensor_tensor(
                            anat[:, h, :], pav[:, 0:64],
                            rec[:, h, :].to_broadcast((P, 64)), mybir.AluOpType.mult,
                        )
                    anat_cells[(qc, pair, qs)] = anat
                return run

            def c2_unit(qc, pair, qs):
                def run():
                    anat = anat_cells.pop((qc, pair, qs))
                    ptr = psC.tile([P, 2 * QCH], fp16, tag="c", name="tr")
                    nc.tensor.transpose(
                        ptr[:, 0:P], anat[:].rearrange("p a b -> p (a b)"), id_sb[:]
                    )
                    tok0 = qc * QCH + qs * P
                    nc.vector.tensor_copy(attnT[:, pair, tok0 : tok0 + P], ptr[:, 0:P])
                return run

            def p3_unit(qc, ts, ec):
                def run():
                    if ec == 0:
                        y_cells[ts] = ypool.tile([P, C], fp16, tag="y", name="ysb")
                    y_sb = y_cells[ts]
                    if qc == 3 and ec % 2 == 1:
                        py = psAV.tile([P, QCH], f32, tag="av", name="py")
                    else:
                        py = psC.tile([P, QCH], f32, tag="c", name="py")
                    esl = slice(ec * QCH, (ec + 1) * QCH)
                    for ks in range(4):
                        nc.tensor.matmul(
                            py[:], attnT[:, ks, ts * P : (ts + 1) * P], wo_sb[:, ks, esl],
                            start=(ks == 0), stop=(ks == 3),
                        )
                    if qc == 3:
                        nc.scalar.activation(
                            y_sb[:, esl], py[:],
                            mybir.ActivationFunctionType.Copy, scale=1.0,
                        )
                        nc.sync.dma_start(y.ap()[ts * P : (ts + 1) * P, esl], y_sb[:, esl])
                    else:
                        nc.vector.tensor_copy(y_sb[:, esl], py[:])
                        nc.gpsimd.dma_start(y.ap()[ts * P : (ts + 1) * P, esl], y_sb[:, esl])
                    if ec == 3:
                        del y_cells[ts]
                return run

            # ---- attention loop ----
            QC_ORDER = [0, 1, 2, 3]
            WARM_KEEP = {0: 0, 1: 4, 2: 16, 3: 0}
            COLD_CAP = {0: 28, 1: 56, 2: 10**9, 3: 10**9}
            for qc in QC_ORDER:
                gate_chunk(min(2 * qc + 1, NCH - 1))
                q0 = qc * QCH
                nfull = 4 * qc
                ntiles = nfull + 4
                for pair in range(NPAIR):
                    for i in range(ntiles):
                        if i < nfull:
                            nsl = slice(0, QCH)
                        else:
                            nsl = slice((i - nfull) * P, QCH)
                        ksl = slice(i * P, (i + 1) * P)
                        qsl = slice(q0 + nsl.start, q0 + nsl.stop)
                        ps_s = psS.tile([P, 2, QCH], f32, tag="s")
                        nc.tensor.matmul(
                            ps_s[:, 0, nsl], qk_sb[0:64, 4, ksl], qk_sb[0:64, pair, qsl],
                            start=True, stop=True,
                        )
                        nc.tensor.matmul(
                            ps_s[:, 1, nsl], qk_sb[64:128, 4, ksl], qk_sb[64:128, pair, qsl],
                            start=True, stop=True,
                        )
                        expS = epool.tile([P, 2, QCH], fp16, tag="expS")
                        nc.scalar.activation(
                            expS[:, :, nsl], ps_s[:, :, nsl],
                            mybir.ActivationFunctionType.Exp, scale=0.125,
                        )
                        if i >= nfull:
                            j = i - nfull
                            nc.vector.tensor_tensor(
                                expS[:, :, j * P : (j + 1) * P],
                                expS[:, :, j * P : (j + 1) * P],
                                mask_sb[:, None, :].to_broadcast((P, 2, P)),
                                mybir.AluOpType.mult,
                            )
                        exp_tiles[(qc, pair, i)] = expS
                        nexp = 2 * (nsl.stop - nsl.start)
                        cap = COLD_CAP[qc]
                        keep = WARM_KEEP[qc]
                        drain(int(nexp * 0.50) + 285, cap, keep)
                    for qs in range(4):
                        nfq = 4 * qc + qs + 1
                        hot.append((int(nfq * 2 * 65 * 0.417), b_unit(qc, pair, qs)))
                        hot.append((120, c1_unit(qc, pair, qs)))
                        if qs >= 1:
                            hot.append((80, c2_unit(qc, pair, qs - 1)))
                    hot.append((80, c2_unit(qc, pair, 3)))
                for ts in range(qc * 4, (qc + 1) * 4):
                    for ec in range(4):
                        warm.append((880, p3_unit(qc, ts, ec)))
            while hot or warm or cold:
                drain(10**9)

    nc.compile()
    return nc


def _prep_inputs(x, Wq, bq, Wk, bk, Wv, bv, Wo, bo):
    x = np.asarray(x, dtype=np.float32)
    Wq = np.asarray(Wq, dtype=np.float32)
    Wk = np.asarray(Wk, dtype=np.float32)
    Wv = np.asarray(Wv, dtype=np.float32)
    Wo = np.asarray(Wo, dtype=np.float32)
    bq = np.asarray(bq, dtype=np.float32)
    bk = np.asarray(bk, dtype=np.float32)
    bv = np.asarray(bv, dtype=np.float32)

    mask = np.triu(np.ones((P, P), dtype=np.float16))
    ident = np.eye(P, dtype=np.float16)

    def tile_k(w):
        return np.ascontiguousarray(
            w.reshape(KT, P, -1).transpose(1, 0, 2).astype(np.float16)
        )

    xt_all = [tile_k(x[b].T.copy()) for b in range(B)]

    in_maps = []
    for c in range(NCORES):
        b, g = c // 4, c % 4
        wq_cols = []
        bq_cols = []
        for p in range(4):
            lo, hi = 8 * g + p, 8 * g + 4 + p
            wq_cols.append(Wq[:, lo * HD : (lo + 1) * HD])
            wq_cols.append(Wq[:, hi * HD : (hi + 1) * HD])
            bq_cols.append(
                np.concatenate([bq[lo * HD : (lo + 1) * HD], bq[hi * HD : (hi + 1) * HD]])
            )
        wq_c = np.concatenate(wq_cols, axis=1)
        kv0, kv1 = 2 * g, 2 * g + 1
        wk_c = Wk[:, kv0 * HD : (kv1 + 1) * HD]
        wv_c = Wv[:, kv0 * HD : (kv1 + 1) * HD]
        bk_c = np.concatenate([bk[kv0 * HD : (kv0 + 1) * HD], bk[kv1 * HD : (kv1 + 1) * HD]])
        bqk_c = np.stack(bq_cols + [bk_c], axis=1)
        wo_rows = []
        for p in range(4):
            lo, hi = 8 * g + p, 8 * g + 4 + p
            wo_rows.append(Wo[lo * HD : (lo + 1) * HD, :])
            wo_rows.append(Wo[hi * HD : (hi + 1) * HD, :])
        wo_c = np.concatenate(wo_rows, axis=0)
        wo_t = np.ascontiguousarray(
            wo_c.reshape(4, P, C).transpose(1, 0, 2).astype(np.float16)
        )
        in_maps.append(
            {
                "xt": xt_all[b],
                "wq": tile_k(wq_c),
                "wk": tile_k(wk_c),
                "wv": tile_k(wv_c),
                "wo": wo_t,
                "bqk": np.ascontiguousarray(bqk_c.astype(np.float32)),
                "bv": np.ascontiguousarray(bv[None, kv0 * HD : (kv1 + 1) * HD]),
                "mask": mask,
                "ident": ident,
            }
        )
    return in_maps


def kernel(x, Wq, bq, Wk, bk, Wv, bv, Wo, bo, _trace=False):
    if not _trace:
        os.environ["BASS_NEVER_TRACE"] = "1"
    if "nc" not in _CACHE:
        _CACHE["nc"] = _build()
    nc = _CACHE["nc"]
    in_maps = _prep_inputs(x, Wq, bq, Wk, bk, Wv, bv, Wo, bo)
    res = bass_utils.run_bass_kernel_spmd(
        nc, in_maps, core_ids=list(range(NCORES)), trace=_trace
    )
    bo = np.asarray(bo, dtype=np.float32)
    y = np.zeros((B, T, C), dtype=np.float32)
    for c in range(NCORES):
        y[c // 4] += res.results[c]["y"].astype(np.float32)
    y += bo
    if _trace:
        return y, res
    return y


# revision 3
# speedup vs baseline: 1.0153x; 1.0094x over previous
"""GQA forward kernel for Trainium2, 8-core (batch x head-quarter) parallel.

Problem: B=2, T=2048, D=2048, 32 Q heads / 8 KV heads, head_dim 64, causal.

Sharding: core c = (batch b = c//4, group g = c%4) owns batch b and head
group g: q heads 8g..8g+7 (as 4 pairs p = heads 8g+p | 8g+4+p), kv heads
2g, 2g+1.  Each core emits a full [T, C] fp16 partial output (row-parallel
over its 512 attn channels); host sums 4 partials per batch (+ bo).

All-fp16 matmul path (fp32 PSUM accumulation):
  P1 projections -> Q^T pair-subs [128,T], K^T [128,T], V natural [T,2,64].
  P2 scores S^T[kv,q] per pair: two K=64 matmuls on partition halves into
     separate PSUM banks, one wide exp (ACT, scale 1/8, no max-subtract),
     causal via column-sliced matmuls + mask multiply on diagonal tiles.
  AV natural: [q,65] += expS.T @ [V|1] per head (denominator = 65th col),
     reciprocal + free-broadcast multiply (DVE), PE transpose (fp16
     identity) -> attn^T.
  P3 y = attn^T.T @ Wo_c (4 k-subs), fp16 y, per-512-col DMA.

Scheduling: PE is in-order, and phase A (scores+exp) is ACT-bound, so all
other PE work is emitted as units drained between score matmuls:
  hot queue  - AV/normalize/transpose/out-proj units (these recycle the
               expS/pav/psC rings; draining them promptly is required for
               correctness of ring reuse ordering - a cold-starved hot
               queue can deadlock the in-order PE stream)
  cold queue - P1 half-chain units for token chunks 2..7 (attention for
               q-chunk qc only needs projections through chunk 2qc+1,
               which gate_chunk() enforces before each A phase)
Each A step drains >=1 hot unit plus enough units to fill the ACT-PE gap.
"""

import os
from collections import deque

import numpy as np

import concourse.mybir as mybir
import concourse.tile as tile
from concourse import bacc
from concourse import bass_utils

P = 128
B = 2
T = 2048
C = 2048
HD = 64
NCORES = 8
NPAIR = 4
TCH = 256
KT = C // P            # 16 contraction tiles
QCH = 512
NQC = T // QCH         # 4 q chunks
NTT = T // P           # 16 token tiles
NCH = T // TCH         # 8 projection chunks
f32 = mybir.dt.float32
fp16 = mybir.dt.float16

_CACHE = {}


def _build():
    nc = bacc.Bacc("TRN2", target_bir_lowering=False, debug=False, num_devices=NCORES)

    xt = nc.dram_tensor("xt", [P, KT, T], fp16, kind="ExternalInput")
    wq = nc.dram_tensor("wq", [P, KT, 4 * P], fp16, kind="ExternalInput")
    wk = nc.dram_tensor("wk", [P, KT, P], fp16, kind="ExternalInput")
    wv = nc.dram_tensor("wv", [P, KT, P], fp16, kind="ExternalInput")
    wo = nc.dram_tensor("wo", [P, 4, C], fp16, kind="ExternalInput")
    bqk = nc.dram_tensor("bqk", [P, 5], f32, kind="ExternalInput")
    bv = nc.dram_tensor("bv", [1, P], f32, kind="ExternalInput")
    maskd = nc.dram_tensor("mask", [P, P], fp16, kind="ExternalInput")
    identd = nc.dram_tensor("ident", [P, P], fp16, kind="ExternalInput")
    y = nc.dram_tensor("y", [T, C], fp16, kind="ExternalOutput")

    with tile.TileContext(nc) as tc:
        with (
            tc.tile_pool(name="const", bufs=1) as cpool,
            tc.tile_pool(name="x", bufs=6) as xpool,
            tc.tile_pool(name="proj", bufs=1) as projpool,
            tc.tile_pool(name="exp", bufs=30) as epool,
            tc.tile_pool(name="work", bufs=6) as wpool,
            tc.tile_pool(name="y", bufs=3) as ypool,
            tc.tile_pool(name="psS", bufs=2, space="PSUM") as psS,
            tc.tile_pool(name="psAV", bufs=2, space="PSUM") as psAV,
            tc.tile_pool(name="psC", bufs=2, space="PSUM") as psC,
        ):
            # ---- DMA schedule: startup-critical quarter interleave ----
            wq_sb = cpool.tile([P, KT, 4 * P], fp16)
            xchs = [
                xpool.tile([P, KT, TCH], fp16, tag="xch", name=f"xch{i}")
                for i in range(NCH)
            ]
            qk_sb = projpool.tile([P, 5, T], fp16, tag="qk")
            v2_sb = projpool.tile([P, NTT, 2, 65], fp16, tag="v2")
            attnT = projpool.tile([P, NPAIR, T], fp16, tag="attnT")
            nc.gpsimd.memset(v2_sb[:, :, :, 64:65], 1.0)
            # SP queue: weights/biases (HWDGE); Pool queue: x/consts/wo (SWDGE)
            bqk_sb = cpool.tile([P, 5], f32)
            wk_sb = cpool.tile([P, KT, P], fp16)
            wv_sb = cpool.tile([P, KT, P], fp16)
            bv_sb = cpool.tile([P, P], f32)
            for q in range(4):
                kq = slice(4 * q, 4 * (q + 1))
                nc.sync.dma_start(wq_sb[:, kq, :], wq.ap()[:, kq, :])
                nc.gpsimd.dma_start(xchs[0][:, kq, :], xt.ap()[:, kq, 0:TCH])
                if q == 1:
                    nc.sync.dma_start(bqk_sb[:], bqk.ap())
                    nc.sync.dma_start(wk_sb[:], wk.ap())
                    nc.sync.dma_start(wv_sb[:], wv.ap())
            nc.sync.dma_start(bv_sb[:], bv.ap().to_broadcast((P, P)))
            mask_sb = cpool.tile([P, P], fp16)
            nc.gpsimd.dma_start(mask_sb[:], maskd.ap())
            id_sb = cpool.tile([P, P], fp16)
            nc.gpsimd.dma_start(id_sb[:], identd.ap())
            wo_sb = cpool.tile([P, 4, C], fp16)
            tsl1 = slice(TCH, 2 * TCH)
            nc.scalar.dma_start(xchs[1][:, 0 : KT // 2, :], xt.ap()[:, 0 : KT // 2, tsl1])
            nc.scalar.dma_start(xchs[1][:, KT // 2 :, :], xt.ap()[:, KT // 2 :, tsl1])
            for i in list(range(2, 6)) + [-1, 6, 7]:
                if i == -1:
                    nc.gpsimd.dma_start(wo_sb[:], wo.ap())
                    continue
                tsl = slice(i * TCH, (i + 1) * TCH)
                nc.gpsimd.dma_start(xchs[i][:, 0 : KT // 2, :], xt.ap()[:, 0 : KT // 2, tsl])
                nc.gpsimd.dma_start(xchs[i][:, KT // 2 :, :], xt.ap()[:, KT // 2 :, tsl])

            # ---- P1 unit emitters ----
            chain_cells = {}

            def p1_sub_half(tch, sub, half):
                def run():
                    xch = xchs[tch]
                    if half == 0:
                        pp = psC.tile([P, QCH], f32, tag="c", name="pp")
                        chain_cells[(tch, sub)] = pp
                    else:
                        pp = chain_cells.pop((tch, sub))
                    for k in range(half * (KT // 2), (half + 1) * (KT // 2)):
                        lhsT = (
                            wq_sb[:, k, sub * P : (sub + 1) * P]
                            if sub < 4
                            else wk_sb[:, k, :]
                        )
                        nc.tensor.matmul(
                            pp[:, 0:TCH], lhsT, xch[:, k, :],
                            start=(k == 0), stop=(k == KT - 1),
                        )
                    if half == 1:
                        tsl = slice(tch * TCH, (tch + 1) * TCH)
                        nc.vector.tensor_tensor(
                            qk_sb[:, sub, tsl], pp[:, 0:TCH],
                            bqk_sb[:, sub : sub + 1].to_broadcast((P, TCH)),
                            mybir.AluOpType.add,
                        )
                return run

            def p1_v_half(tch, ts2, half):
                def run():
                    xch = xchs[tch]
                    tidx = tch * (TCH // P) + ts2
                    if half == 0:
                        pv = psC.tile([P, QCH], f32, tag="c", name="pv")
                        chain_cells[(tch, "v", ts2)] = pv
                    else:
                        pv = chain_cells.pop((tch, "v", ts2))
                    for k in range(half * (KT // 2), (half + 1) * (KT // 2)):
                        nc.tensor.matmul(
                            pv[:, 0:P], xch[:, k, ts2 * P : (ts2 + 1) * P], wv_sb[:, k, :],
                            start=(k == 0), stop=(k == KT - 1),
                        )
                    if half == 1:
                        nc.vector.tensor_tensor(
                            v2_sb[:, tidx, :, 0:64],
                            pv[:, 0:P].rearrange("p (h d) -> p h d", h=2),
                            bv_sb[:].rearrange("p (h d) -> p h d", h=2),
                            mybir.AluOpType.add,
                        )
                return run

            def p1_chunk_units(tch):
                units = []
                for sub in range(5):
                    units.append((853, p1_sub_half(tch, sub, 0)))
                    units.append((853, p1_sub_half(tch, sub, 1)))
                for ts2 in range(TCH // P):
                    units.append((427, p1_v_half(tch, ts2, 0)))
                    units.append((427, p1_v_half(tch, ts2, 1)))
                return units

            # chunks 0,1 inline (needed before any attention)
            for tch in range(2):
                for _, u in p1_chunk_units(tch):
                    u()

            hot = deque()
            warm = deque()
            cold = deque()
            chunk_done = 1  # highest P1 chunk fully emitted
            for tch in range(2, NCH):
                for cost, u in p1_chunk_units(tch):
                    cold.append((cost, u))

            cold_popped = [0]
            cold_total = {tch: 14 * (tch - 1) for tch in range(2, NCH)}

            def pop_cold():
                cost, u = cold.popleft()
                cold_popped[0] += 1
                u()
                return cost

            def gate_chunk(tch):
                # force-drain cold until chunk tch fully emitted
                nonlocal chunk_done
                if tch <= chunk_done:
                    return
                need = cold_total[min(tch, NCH - 1)] - cold_popped[0]
                for _ in range(max(0, need)):
                    pop_cold()
                chunk_done = max(chunk_done, tch)

            def drain(budget, cold_cap=10**9, warm_keep=0):
                while hot and budget > 0:
                    cost, u = hot.popleft()
                    u()
                    budget -= cost
                while warm and budget > 0 and len(warm) > warm_keep:
                    cost, u = warm.popleft()
                    u()
                    budget -= cost
                while cold and budget > 0 and cold_popped[0] < cold_cap:
                    budget -= pop_cold()

            # ---- attention unit emitters ----
            exp_tiles = {}
            pav_cells = {}
            anat_cells = {}
            y_cells = {}

            def b_unit(qc, pair, qs):
                def run():
                    nfull = 4 * qc
                    last = nfull + qs
                    pav0 = psAV.tile([P, QCH], f32, tag="av", name="pav0")
                    pav1 = psAV.tile([P, QCH], f32, tag="av", name="pav1")
                    qsl = slice(qs * P, (qs + 1) * P)
                    for i in range(last + 1):
                        e = exp_tiles[(qc, pair, i)]
                        nc.tensor.matmul(
                            pav0[:, 0:65], e[:, 0, qsl], v2_sb[:, i, 0, :],
                            start=(i == 0), stop=(i == last), skip_group_check=True,
                        )
                        nc.tensor.matmul(
                            pav1[:, 0:65], e[:, 1, qsl], v2_sb[:, i, 1, :],
                            start=(i == 0), stop=(i == last), skip_group_check=True,
                        )
                    pav_cells[(qc, pair, qs)] = (pav0, pav1)
                return run

            def c1_unit(qc, pair, qs):
                def run():
                    pav0, pav1 = pav_cells.pop((qc, pair, qs))
                    rec = wpool.tile([P, 2, 1], f32, tag="rec")
                    anat = wpool.tile([P, 2, 64], fp16, tag="anat")
                    for h, pav in ((0, pav0), (1, pav1)):
                        nc.vector.reciprocal(rec[:, h, :], pav[:, 64:65])
                        nc.vector.tensor_tensor(
                            anat[:, h, :], pav[:, 0:64],
                            rec[:, h, :].to_broadcast((P, 64)), mybir.AluOpType.mult,
                        )
                    anat_cells[(qc, pair, qs)] = anat
                return run

            def c2_unit(qc, pair, qs):
                def run():
                    anat = anat_cells.pop((qc, pair, qs))
                    ptr = psC.tile([P, 2 * QCH], fp16, tag="c", name="tr")
                    nc.tensor.transpose(
                        ptr[:, 0:P], anat[:].rearrange("p a b -> p (a b)"), id_sb[:]
                    )
                    tok0 = qc * QCH + qs * P
                    nc.vector.tensor_copy(attnT[:, pair, tok0 : tok0 + P], ptr[:, 0:P])
                return run

            def p3_unit(qc, ts, ec):
                def run():
                    if ec == 0:
                        y_cells[ts] = ypool.tile([P, C], fp16, tag="y", name="ysb")
                    y_sb = y_cells[ts]
                    if qc == 3:
                        r = (ts * 4 + ec) % 3
                        if r == 0:
                            py = psC.tile([P, QCH], f32, tag="c", name="py")
                        elif r == 1:
                            py = psAV.tile([P, QCH], f32, tag="av", name="py")
                        else:
                            py = psS.tile([P, QCH], f32, tag="s", name="py")
                    else:
                        py = psC.tile([P, QCH], f32, tag="c", name="py")
                    esl = slice(ec * QCH, (ec + 1) * QCH)
                    for ks in range(4):
                        nc.tensor.matmul(
                            py[:], attnT[:, ks, ts * P : (ts + 1) * P], wo_sb[:, ks, esl],
                            start=(ks == 0), stop=(ks == 3),
                        )
                    if qc == 3:
                        nc.scalar.activation(
                            y_sb[:, esl], py[:],
                            mybir.ActivationFunctionType.Copy, scale=1.0,
                        )
                        nc.sync.dma_start(y.ap()[ts * P : (ts + 1) * P, esl], y_sb[:, esl])
                    else:
                        nc.vector.tensor_copy(y_sb[:, esl], py[:])
                        nc.gpsimd.dma_start(y.ap()[ts * P : (ts + 1) * P, esl], y_sb[:, esl])
                    if ec == 3:
                        del y_cells[ts]
                return run

            # ---- attention loop ----
            QC_ORDER = [0, 1, 2, 3]
            WARM_KEEP = {0: 0, 1: 4, 2: 16, 3: 0}
            COLD_CAP = {0: 28, 1: 56, 2: 10**9, 3: 10**9}
            for qc in QC_ORDER:
                gate_chunk(min(2 * qc + 1, NCH - 1))
                q0 = qc * QCH
                nfull = 4 * qc
                ntiles = nfull + 4
                for pair in range(NPAIR):
                    for i in range(ntiles):
                        if i < nfull:
                            nsl = slice(0, QCH)
                        else:
                            nsl = slice((i - nfull) * P, QCH)
                        ksl = slice(i * P, (i + 1) * P)
                        qsl = slice(q0 + nsl.start, q0 + nsl.stop)
                        ps_s = psS.tile([P, 2, QCH], f32, tag="s")
                        nc.tensor.matmul(
                            ps_s[:, 0, nsl], qk_sb[0:64, 4, ksl], qk_sb[0:64, pair, qsl],
                            start=True, stop=True,
                        )
                        nc.tensor.matmul(
                            ps_s[:, 1, nsl], qk_sb[64:128, 4, ksl], qk_sb[64:128, pair, qsl],
                            start=True, stop=True,
                        )
                        expS = epool.tile([P, 2, QCH], fp16, tag="expS")
                        nc.scalar.activation(
                            expS[:, :, nsl], ps_s[:, :, nsl],
                            mybir.ActivationFunctionType.Exp, scale=0.125,
                        )
                        if i >= nfull:
                            j = i - nfull
                            nc.vector.tensor_tensor(
                                expS[:, :, j * P : (j + 1) * P],
                                expS[:, :, j * P : (j + 1) * P],
                                mask_sb[:, None, :].to_broadcast((P, 2, P)),
                                mybir.AluOpType.mult,
                            )
                        exp_tiles[(qc, pair, i)] = expS
                        nexp = 2 * (nsl.stop - nsl.start)
                        cap = COLD_CAP[qc]
                        keep = WARM_KEEP[qc]
                        drain(int(nexp * 0.50) + 285, cap, keep)
                    for qs in range(4):
                        nfq = 4 * qc + qs + 1
                        hot.append((int(nfq * 2 * 65 * 0.417), b_unit(qc, pair, qs)))
                        hot.append((120, c1_unit(qc, pair, qs)))
                        if qs >= 1:
                            hot.append((80, c2_unit(qc, pair, qs - 1)))
                    hot.append((80, c2_unit(qc, pair, 3)))
                for ts in range(qc * 4, (qc + 1) * 4):
                    for ec in range(4):
                        warm.append((880, p3_unit(qc, ts, ec)))
            while hot or warm or cold:
                drain(10**9)

    nc.compile()
    return nc


def _prep_inputs(x, Wq, bq, Wk, bk, Wv, bv, Wo, bo):
    x = np.asarray(x, dtype=np.float32)
    Wq = np.asarray(Wq, dtype=np.float32)
    Wk = np.asarray(Wk, dtype=np.float32)
    Wv = np.asarray(Wv, dtype=np.float32)
    Wo = np.asarray(Wo, dtype=np.float32)
    bq = np.asarray(bq, dtype=np.float32)
    bk = np.asarray(bk, dtype=np.float32)
    bv = np.asarray(bv, dtype=np.float32)

    mask = np.triu(np.ones((P, P), dtype=np.float16))
    ident = np.eye(P, dtype=np.float16)

    def tile_k(w):
        return np.ascontiguousarray(
            w.reshape(KT, P, -1).transpose(1, 0, 2).astype(np.float16)
        )

    xt_all = [tile_k(x[b].T.copy()) for b in range(B)]

    in_maps = []
    for c in range(NCORES):
        b, g = c // 4, c % 4
        wq_cols = []
        bq_cols = []
        for p in range(4):
            lo, hi = 8 * g + p, 8 * g + 4 + p
            wq_cols.append(Wq[:, lo * HD : (lo + 1) * HD])
            wq_cols.append(Wq[:, hi * HD : (hi + 1) * HD])
            bq_cols.append(
                np.concatenate([bq[lo * HD : (lo + 1) * HD], bq[hi * HD : (hi + 1) * HD]])
            )
        wq_c = np.concatenate(wq_cols, axis=1)
        kv0, kv1 = 2 * g, 2 * g + 1
        wk_c = Wk[:, kv0 * HD : (kv1 + 1) * HD]
        wv_c = Wv[:, kv0 * HD : (kv1 + 1) * HD]
        bk_c = np.concatenate([bk[kv0 * HD : (kv0 + 1) * HD], bk[kv1 * HD : (kv1 + 1) * HD]])
        bqk_c = np.stack(bq_cols + [bk_c], axis=1)
        wo_rows = []
        for p in range(4):
            lo, hi = 8 * g + p, 8 * g + 4 + p
            wo_rows.append(Wo[lo * HD : (lo + 1) * HD, :])
            wo_rows.append(Wo[hi * HD : (hi + 1) * HD, :])
        wo_c = np.concatenate(wo_rows, axis=0)
        wo_t = np.ascontiguousarray(
            wo_c.reshape(4, P, C).transpose(1, 0, 2).astype(np.float16)
        )
        in_maps.append(
            {
                "xt": xt_all[b],
                "wq": tile_k(wq_c),
                "wk": tile_k(wk_c),
                "wv": tile_k(wv_c),
                "wo": wo_t,
                "bqk": np.ascontiguousarray(bqk_c.astype(np.float32)),
                "bv": np.ascontiguousarray(bv[None, kv0 * HD : (kv1 + 1) * HD]),
                "mask": mask,
                "ident": ident,
            }
        )
    return in_maps


def kernel(x, Wq, bq, Wk, bk, Wv, bv, Wo, bo, _trace=False):
    if not _trace:
        os.environ["BASS_NEVER_TRACE"] = "1"
    if "nc" not in _CACHE:
        _CACHE["nc"] = _build()
    nc = _CACHE["nc"]
    in_maps = _prep_inputs(x, Wq, bq, Wk, bk, Wv, bv, Wo, bo)
    res = bass_utils.run_bass_kernel_spmd(
        nc, in_maps, core_ids=list(range(NCORES)), trace=_trace
    )
    bo = np.asarray(bo, dtype=np.float32)
    y = np.zeros((B, T, C), dtype=np.float32)
    for c in range(NCORES):
        y[c // 4] += res.results[c]["y"].astype(np.float32)
    y += bo
    if _trace:
        return y, res
    return y


# revision 6
# speedup vs baseline: 1.0407x; 1.0250x over previous
"""GQA forward kernel for Trainium2, 8-core (batch x head-quarter) parallel.

Problem: B=2, T=2048, D=2048, 32 Q heads / 8 KV heads, head_dim 64, causal.

Sharding: core c = (batch b = c//4, group g = c%4) owns batch b and head
group g: q heads 8g..8g+7 (as 4 pairs p = heads 8g+p | 8g+4+p), kv heads
2g, 2g+1.  Each core emits a full [T, C] fp16 partial output (row-parallel
over its 512 attn channels); host sums 4 partials per batch (+ bo).

All-fp16 matmul path (fp32 PSUM accumulation):
  P1 projections -> Q^T pair-subs [128,T], K^T [128,T], V natural [T,2,64].
  P2 scores S^T[kv,q] per pair: two K=64 matmuls on partition halves into
     separate PSUM banks, one wide exp (ACT, scale 1/8, no max-subtract),
     causal via column-sliced matmuls + mask multiply on diagonal tiles.
  AV natural: [q,65] += expS.T @ [V|1] per head (denominator = 65th col),
     reciprocal + free-broadcast multiply (DVE), PE transpose (fp16
     identity) -> attn^T.
  P3 y = attn^T.T @ Wo_c (4 k-subs), fp16 y, per-512-col DMA.

Scheduling: PE is in-order, and phase A (scores+exp) is ACT-bound, so all
other PE work is emitted as units drained between score matmuls:
  hot queue  - AV/normalize/transpose/out-proj units (these recycle the
               expS/pav/psC rings; draining them promptly is required for
               correctness of ring reuse ordering - a cold-starved hot
               queue can deadlock the in-order PE stream)
  cold queue - P1 half-chain units for token chunks 2..7 (attention for
               q-chunk qc only needs projections through chunk 2qc+1,
               which gate_chunk() enforces before each A phase)
Each A step drains >=1 hot unit plus enough units to fill the ACT-PE gap.
"""

import os
from collections import deque

import numpy as np

import concourse.mybir as mybir
import concourse.tile as tile
from concourse import bacc
from concourse import bass_utils

P = 128
B = 2
T = 2048
C = 2048
HD = 64
NCORES = 8
NPAIR = 4
TCH = 256
KT = C // P            # 16 contraction tiles
QCH = 512
NQC = T // QCH         # 4 q chunks
NTT = T // P           # 16 token tiles
NCH = T // TCH         # 8 projection chunks
f32 = mybir.dt.float32
fp16 = mybir.dt.float16

_CACHE = {}


def _build():
    nc = bacc.Bacc("TRN2", target_bir_lowering=False, debug=False, num_devices=NCORES)

    xt = nc.dram_tensor("xt", [P, KT, T], fp16, kind="ExternalInput")
    wq = nc.dram_tensor("wq", [P, KT, 4 * P], fp16, kind="ExternalInput")
    wk = nc.dram_tensor("wk", [P, KT, P], fp16, kind="ExternalInput")
    wv = nc.dram_tensor("wv", [P, KT, P], fp16, kind="ExternalInput")
    wo = nc.dram_tensor("wo", [P, 4, C], fp16, kind="ExternalInput")
    bqk = nc.dram_tensor("bqk", [P, 5], f32, kind="ExternalInput")
    bv = nc.dram_tensor("bv", [1, P], f32, kind="ExternalInput")
    maskd = nc.dram_tensor("mask", [P, P], fp16, kind="ExternalInput")
    identd = nc.dram_tensor("ident", [P, P], fp16, kind="ExternalInput")
    y = nc.dram_tensor("y", [T, C], fp16, kind="ExternalOutput")

    with tile.TileContext(nc) as tc:
        with (
            tc.tile_pool(name="const", bufs=1) as cpool,
            tc.tile_pool(name="x", bufs=6) as xpool,
            tc.tile_pool(name="proj", bufs=1) as projpool,
            tc.tile_pool(name="exp", bufs=28) as epool,
            tc.tile_pool(name="work", bufs=6) as wpool,
            tc.tile_pool(name="y", bufs=3) as ypool,
            tc.tile_pool(name="psS", bufs=2, space="PSUM") as psS,
            tc.tile_pool(name="psAV", bufs=2, space="PSUM") as psAV,
            tc.tile_pool(name="psC", bufs=2, space="PSUM") as psC,
        ):
            # ---- DMA schedule: startup-critical quarter interleave ----
            wq_sb = cpool.tile([P, KT, 4 * P], fp16)
            xchs = [
                xpool.tile([P, KT, TCH], fp16, tag="xch", name=f"xch{i}")
                for i in range(NCH)
            ]
            qk_sb = projpool.tile([P, 5, T], fp16, tag="qk")
            v2_sb = projpool.tile([P, NTT, 2, 65], fp16, tag="v2")
            attnT = projpool.tile([P, NPAIR, T], fp16, tag="attnT")
            nc.gpsimd.memset(v2_sb[:, :, :, 64:65], 1.0)
            # SP queue: weights/biases (HWDGE); Pool queue: x/consts/wo (SWDGE)
            bqk_sb = cpool.tile([P, 5], f32)
            wk_sb = cpool.tile([P, KT, P], fp16)
            wv_sb = cpool.tile([P, KT, P], fp16)
            bv_sb = cpool.tile([P, P], f32)
            for q in range(4):
                kq = slice(4 * q, 4 * (q + 1))
                nc.sync.dma_start(wq_sb[:, kq, :], wq.ap()[:, kq, :])
                nc.gpsimd.dma_start(xchs[0][:, kq, :], xt.ap()[:, kq, 0:TCH])
                if q == 1:
                    nc.sync.dma_start(bqk_sb[:], bqk.ap())
                    nc.sync.dma_start(wk_sb[:], wk.ap())
                    nc.sync.dma_start(wv_sb[:], wv.ap())
            nc.sync.dma_start(bv_sb[:], bv.ap().to_broadcast((P, P)))
            mask_sb = cpool.tile([P, P], fp16)
            nc.gpsimd.dma_start(mask_sb[:], maskd.ap())
            id_sb = cpool.tile([P, P], fp16)
            nc.gpsimd.dma_start(id_sb[:], identd.ap())
            wo_sb = cpool.tile([P, 4, C], fp16)
            tsl1 = slice(TCH, 2 * TCH)
            nc.scalar.dma_start(xchs[1][:, 0 : KT // 2, :], xt.ap()[:, 0 : KT // 2, tsl1])
            nc.scalar.dma_start(xchs[1][:, KT // 2 :, :], xt.ap()[:, KT // 2 :, tsl1])
            for i in list(range(2, 6)) + [-1, 6, 7]:
                if i == -1:
                    nc.gpsimd.dma_start(wo_sb[:], wo.ap())
                    continue
                tsl = slice(i * TCH, (i + 1) * TCH)
                nc.gpsimd.dma_start(xchs[i][:, 0 : KT // 2, :], xt.ap()[:, 0 : KT // 2, tsl])
                nc.gpsimd.dma_start(xchs[i][:, KT // 2 :, :], xt.ap()[:, KT // 2 :, tsl])

            # ---- P1 unit emitters ----
            chain_cells = {}

            def p1_sub_half(tch, sub, half):
                def run():
                    xch = xchs[tch]
                    if half == 0:
                        pp = psC.tile([P, QCH], f32, tag="c", name="pp")
                        chain_cells[(tch, sub)] = pp
                    else:
                        pp = chain_cells.pop((tch, sub))
                    for k in range(half * (KT // 2), (half + 1) * (KT // 2)):
                        lhsT = (
                            wq_sb[:, k, sub * P : (sub + 1) * P]
                            if sub < 4
                            else wk_sb[:, k, :]
                        )
                        nc.tensor.matmul(
                            pp[:, 0:TCH], lhsT, xch[:, k, :],
                            start=(k == 0), stop=(k == KT - 1),
                        )
                    if half == 1:
                        tsl = slice(tch * TCH, (tch + 1) * TCH)
                        nc.vector.tensor_tensor(
                            qk_sb[:, sub, tsl], pp[:, 0:TCH],
                            bqk_sb[:, sub : sub + 1].to_broadcast((P, TCH)),
                            mybir.AluOpType.add,
                        )
                return run

            def p1_v_half(tch, ts2, half):
                def run():
                    xch = xchs[tch]
                    tidx = tch * (TCH // P) + ts2
                    if half == 0:
                        pv = psC.tile([P, QCH], f32, tag="c", name="pv")
                        chain_cells[(tch, "v", ts2)] = pv
                    else:
                        pv = chain_cells.pop((tch, "v", ts2))
                    for k in range(half * (KT // 2), (half + 1) * (KT // 2)):
                        nc.tensor.matmul(
                            pv[:, 0:P], xch[:, k, ts2 * P : (ts2 + 1) * P], wv_sb[:, k, :],
                            start=(k == 0), stop=(k == KT - 1),
                        )
                    if half == 1:
                        nc.vector.tensor_tensor(
                            v2_sb[:, tidx, :, 0:64],
                            pv[:, 0:P].rearrange("p (h d) -> p h d", h=2),
                            bv_sb[:].rearrange("p (h d) -> p h d", h=2),
                            mybir.AluOpType.add,
                        )
                return run

            def p1_chunk_units(tch):
                units = []
                for sub in range(5):
                    units.append((853, p1_sub_half(tch, sub, 0)))
                    units.append((853, p1_sub_half(tch, sub, 1)))
                for ts2 in range(TCH // P):
                    units.append((427, p1_v_half(tch, ts2, 0)))
                    units.append((427, p1_v_half(tch, ts2, 1)))
                return units

            # chunks 0,1 inline (needed before any attention); k-half
            # interleaved across 4 psum slots so PE can run k<8 matmuls of
            # subs 0-3 while the later wq quarters / x halves still load
            for tch in range(2):
                slots = {}
                for sub in range(4):
                    pool = psC if sub < 2 else psAV
                    tag = "c" if sub < 2 else "av"
                    slots[sub] = pool.tile([P, QCH], f32, tag=tag, name="pp")
                xch = xchs[tch]
                for half in range(2):
                    for sub in range(4):
                        pp = slots[sub]
                        for k in range(half * (KT // 2), (half + 1) * (KT // 2)):
                            nc.tensor.matmul(
                                pp[:, 0:TCH], wq_sb[:, k, sub * P : (sub + 1) * P],
                                xch[:, k, :], start=(k == 0), stop=(k == KT - 1),
                            )
                        if half == 1:
                            tsl = slice(tch * TCH, (tch + 1) * TCH)
                            nc.vector.tensor_tensor(
                                qk_sb[:, sub, tsl], pp[:, 0:TCH],
                                bqk_sb[:, sub : sub + 1].to_broadcast((P, TCH)),
                                mybir.AluOpType.add,
                            )
                for _, u in p1_chunk_units(tch)[8:]:
                    u()

            hot = deque()
            warm = deque()
            cold = deque()
            chunk_done = 1  # highest P1 chunk fully emitted
            for tch in range(2, NCH):
                for cost, u in p1_chunk_units(tch):
                    cold.append((cost, u))

            cold_popped = [0]
            cold_total = {tch: 14 * (tch - 1) for tch in range(2, NCH)}

            def pop_cold():
                cost, u = cold.popleft()
                cold_popped[0] += 1
                u()
                return cost

            def gate_chunk(tch):
                # force-drain cold until chunk tch fully emitted
                nonlocal chunk_done
                if tch <= chunk_done:
                    return
                need = cold_total[min(tch, NCH - 1)] - cold_popped[0]
                for _ in range(max(0, need)):
                    pop_cold()
                chunk_done = max(chunk_done, tch)

            def drain(budget, cold_cap=10**9, warm_keep=0):
                while hot and budget > 0:
                    cost, u = hot.popleft()
                    u()
                    budget -= cost
                while warm and budget > 0 and len(warm) > warm_keep:
                    cost, u = warm.popleft()
                    u()
                    budget -= cost
                while cold and budget > 0 and cold_popped[0] < cold_cap:
                    budget -= pop_cold()

            # ---- attention unit emitters ----
            exp_tiles = {}
            pav_cells = {}
            anat_cells = {}
            y_cells = {}

            def b_unit(qc, pair, qs):
                def run():
                    nfull = 4 * qc
                    last = nfull + qs
                    pav0 = psAV.tile([P, QCH], f32, tag="av", name="pav0")
                    pav1 = psAV.tile([P, QCH], f32, tag="av", name="pav1")
                    qsl = slice(qs * P, (qs + 1) * P)
                    for i in range(last + 1):
                        e = exp_tiles[(qc, pair, i)]
                        nc.tensor.matmul(
                            pav0[:, 0:65], e[:, 0, qsl], v2_sb[:, i, 0, :],
                            start=(i == 0), stop=(i == last), skip_group_check=True,
                        )
                        nc.tensor.matmul(
                            pav1[:, 0:65], e[:, 1, qsl], v2_sb[:, i, 1, :],
                            start=(i == 0), stop=(i == last), skip_group_check=True,
                        )
                    pav_cells[(qc, pair, qs)] = (pav0, pav1)
                return run

            def c1_unit(qc, pair, qs):
                def run():
                    pav0, pav1 = pav_cells.pop((qc, pair, qs))
                    rec = wpool.tile([P, 2, 1], f32, tag="rec")
                    anat = wpool.tile([P, 2, 64], fp16, tag="anat")
                    for h, pav in ((0, pav0), (1, pav1)):
                        nc.vector.reciprocal(rec[:, h, :], pav[:, 64:65])
                        nc.vector.tensor_tensor(
                            anat[:, h, :], pav[:, 0:64],
                            rec[:, h, :].to_broadcast((P, 64)), mybir.AluOpType.mult,
                        )
                    anat_cells[(qc, pair, qs)] = anat
                return run

            def c2_unit(qc, pair, qs):
                def run():
                    anat = anat_cells.pop((qc, pair, qs))
                    ptr = psC.tile([P, 2 * QCH], fp16, tag="c", name="tr")
                    nc.tensor.transpose(
                        ptr[:, 0:P], anat[:].rearrange("p a b -> p (a b)"), id_sb[:]
                    )
                    tok0 = qc * QCH + qs * P
                    nc.vector.tensor_copy(attnT[:, pair, tok0 : tok0 + P], ptr[:, 0:P])
                return run

            def p3_unit(qc, ts, ec):
                def run():
                    if ec == 0:
                        y_cells[ts] = ypool.tile([P, C], fp16, tag="y", name="ysb")
                    y_sb = y_cells[ts]
                    if qc == 3:
                        r = (ts * 4 + ec) % 3
                        if r == 0:
                            py = psC.tile([P, QCH], f32, tag="c", name="py")
                        elif r == 1:
                            py = psAV.tile([P, QCH], f32, tag="av", name="py")
                        else:
                            py = psS.tile([P, QCH], f32, tag="s", name="py")
                    else:
                        py = psC.tile([P, QCH], f32, tag="c", name="py")
                    esl = slice(ec * QCH, (ec + 1) * QCH)
                    for ks in range(4):
                        nc.tensor.matmul(
                            py[:], attnT[:, ks, ts * P : (ts + 1) * P], wo_sb[:, ks, esl],
                            start=(ks == 0), stop=(ks == 3),
                        )
                    if qc == 3:
                        nc.scalar.activation(
                            y_sb[:, esl], py[:],
                            mybir.ActivationFunctionType.Copy, scale=1.0,
                        )
                        nc.sync.dma_start(y.ap()[ts * P : (ts + 1) * P, esl], y_sb[:, esl])
                    else:
                        nc.vector.tensor_copy(y_sb[:, esl], py[:])
                        nc.gpsimd.dma_start(y.ap()[ts * P : (ts + 1) * P, esl], y_sb[:, esl])
                    if ec == 3:
                        del y_cells[ts]
                return run

            # ---- attention loop ----
            QC_ORDER = [0, 1, 2, 3]
            WARM_KEEP = {0: 0, 1: 16, 2: 28, 3: 0}
            COLD_CAP = {0: 28, 1: 56, 2: 10**9, 3: 10**9}
            for qc in QC_ORDER:
                gate_chunk(min(2 * qc + 1, NCH - 1))
                q0 = qc * QCH
                nfull = 4 * qc
                ntiles = nfull + 4
                for pair in range(NPAIR):
                    for i in range(ntiles):
                        if i < nfull:
                            nsl = slice(0, QCH)
                        else:
                            nsl = slice((i - nfull) * P, QCH)
                        ksl = slice(i * P, (i + 1) * P)
                        qsl = slice(q0 + nsl.start, q0 + nsl.stop)
                        ps_s = psS.tile([P, 2, QCH], f32, tag="s")
                        nc.tensor.matmul(
                            ps_s[:, 0, nsl], qk_sb[0:64, 4, ksl], qk_sb[0:64, pair, qsl],
                            start=True, stop=True,
                        )
                        nc.tensor.matmul(
                            ps_s[:, 1, nsl], qk_sb[64:128, 4, ksl], qk_sb[64:128, pair, qsl],
                            start=True, stop=True,
                        )
                        expS = epool.tile([P, 2, QCH], fp16, tag="expS")
                        nc.scalar.activation(
                            expS[:, :, nsl], ps_s[:, :, nsl],
                            mybir.ActivationFunctionType.Exp, scale=0.125,
                        )
                        if i >= nfull:
                            j = i - nfull
                            nc.vector.tensor_tensor(
                                expS[:, :, j * P : (j + 1) * P],
                                expS[:, :, j * P : (j + 1) * P],
                                mask_sb[:, None, :].to_broadcast((P, 2, P)),
                                mybir.AluOpType.mult,
                            )
                        exp_tiles[(qc, pair, i)] = expS
                        nexp = 2 * (nsl.stop - nsl.start)
                        cap = COLD_CAP[qc]
                        keep = WARM_KEEP[qc]
                        drain(int(nexp * 0.50) + 285, cap, keep)
                    for qs in range(4):
                        nfq = 4 * qc + qs + 1
                        hot.append((int(nfq * 2 * 65 * 0.417), b_unit(qc, pair, qs)))
                        hot.append((120, c1_unit(qc, pair, qs)))
                        if qs >= 1:
                            hot.append((80, c2_unit(qc, pair, qs - 1)))
                    hot.append((80, c2_unit(qc, pair, 3)))
                for ts in range(qc * 4, (qc + 1) * 4):
                    for ec in range(4):
                        warm.append((880, p3_unit(qc, ts, ec)))
            while hot or warm or cold:
                drain(10**9)

    nc.compile()
    return nc


def _prep_inputs(x, Wq, bq, Wk, bk, Wv, bv, Wo, bo):
    x = np.asarray(x, dtype=np.float32)
    Wq = np.asarray(Wq, dtype=np.float32)
    Wk = np.asarray(Wk, dtype=np.float32)
    Wv = np.asarray(Wv, dtype=np.float32)
    Wo = np.asarray(Wo, dtype=np.float32)
    bq = np.asarray(bq, dtype=np.float32)
    bk = np.asarray(bk, dtype=np.float32)
    bv = np.asarray(bv, dtype=np.float32)

    mask = np.triu(np.ones((P, P), dtype=np.float16))
    ident = np.eye(P, dtype=np.float16)

    def tile_k(w):
        return np.ascontiguousarray(
            w.reshape(KT, P, -1).transpose(1, 0, 2).astype(np.float16)
        )

    xt_all = [tile_k(x[b].T.copy()) for b in range(B)]

    in_maps = []
    for c in range(NCORES):
        b, g = c // 4, c % 4
        wq_cols = []
        bq_cols = []
        for p in range(4):
            lo, hi = 8 * g + p, 8 * g + 4 + p
            wq_cols.append(Wq[:, lo * HD : (lo + 1) * HD])
            wq_cols.append(Wq[:, hi * HD : (hi + 1) * HD])
            bq_cols.append(
                np.concatenate([bq[lo * HD : (lo + 1) * HD], bq[hi * HD : (hi + 1) * HD]])
            )
        wq_c = np.concatenate(wq_cols, axis=1)
        kv0, kv1 = 2 * g, 2 * g + 1
        wk_c = Wk[:, kv0 * HD : (kv1 + 1) * HD]
        wv_c = Wv[:, kv0 * HD : (kv1 + 1) * HD]
        bk_c = np.concatenate([bk[kv0 * HD : (kv0 + 1) * HD], bk[kv1 * HD : (kv1 + 1) * HD]])
        bqk_c = np.stack(bq_cols + [bk_c], axis=1)
        wo_rows = []
        for p in range(4):
            lo, hi = 8 * g + p, 8 * g + 4 + p
            wo_rows.append(Wo[lo * HD : (lo + 1) * HD, :])
            wo_rows.append(Wo[hi * HD : (hi + 1) * HD, :])
        wo_c = np.concatenate(wo_rows, axis=0)
        wo_t = np.ascontiguousarray(
            wo_c.reshape(4, P, C).transpose(1, 0, 2).astype(np.float16)
        )
        in_maps.append(
            {
                "xt": xt_all[b],
                "wq": tile_k(wq_c),
                "wk": tile_k(wk_c),
                "wv": tile_k(wv_c),
                "wo": wo_t,
                "bqk": np.ascontiguousarray(bqk_c.astype(np.float32)),
                "bv": np.ascontiguousarray(bv[None, kv0 * HD : (kv1 + 1) * HD]),
                "mask": mask,
                "ident": ident,
            }
        )
    return in_maps


def kernel(x, Wq, bq, Wk, bk, Wv, bv, Wo, bo, _trace=False):
    if not _trace:
        os.environ["BASS_NEVER_TRACE"] = "1"
    if "nc" not in _CACHE:
        _CACHE["nc"] = _build()
    nc = _CACHE["nc"]
    in_maps = _prep_inputs(x, Wq, bq, Wk, bk, Wv, bv, Wo, bo)
    res = bass_utils.run_bass_kernel_spmd(
        nc, in_maps, core_ids=list(range(NCORES)), trace=_trace
    )
    bo = np.asarray(bo, dtype=np.float32)
    y = np.zeros((B, T, C), dtype=np.float32)
    for c in range(NCORES):
        y[c // 4] += res.results[c]["y"].astype(np.float32)
    y += bo
    if _trace:
        return y, res
    return y
